# revision 15
# baseline (speedup 1.0000x reference)
"""ResNet BasicBlock (conv3x3-bn-relu-conv3x3-bn-add-relu) on 8 TRN2 cores.

Data-parallel: batch N=64 split into 8 images per core; conv/BN params
replicated. Each 3x3 conv is computed as 9 shifted [128ci x 128co] fp32r
matmuls accumulated in PSUM over a zero-padded [C, 58*58] SBUF image layout
(channels on partitions, padded spatial flattened on the free dim). The host
pre-pads x so each image is one contiguous DMA. Matmul rhs uses strided
valid-column access patterns so no PE cycles are spent on pad columns.
"""

import numpy as np
from contextlib import ExitStack

import concourse.bass as bass
import concourse.bacc as bacc
import concourse.mybir as mybir
from concourse.tile import TileContext
from concourse.bass_utils import run_bass_kernel_spmd

F32 = mybir.dt.float32
F32R = mybir.dt.float32r
BF16 = mybir.dt.bfloat16
RELU = mybir.ActivationFunctionType.Relu
IDENT = mybir.ActivationFunctionType.Identity

N_CORES = 8
N_IMG = 8          # images per core
C = 128            # channels (== partitions)
H = W = 56
HP = WP = 58       # padded spatial
S = HP * WP        # 3364 padded flat size
ALLOC = S + 4      # margins so strided valid-col views stay in bounds
HW = H * W         # 3136
ROW_CHUNKS = [(8 * k, 8) for k in range(7)]  # (start row, rows); 448 <= 512
NMAX = 8 * W


def _valid3(t, start, rows):
    """3D [C, rows, 56] valid-column view of padded tile t at alloc offset
    `start` (the alloc index of the first element of the window)."""
    return t[:, start : start + 58 * rows].rearrange("p (r w) -> p r w", w=58)[
        :, :, 0:56
    ]


def _zero_pads_scalar(nc, t, zc):
    """Zero every padded position of a [128, ALLOC] image tile on ScalarE.

    Implemented as ACT copies from a zero constant tile so the output dtype
    stays float32r (the BIR verifier requires every producer feeding an fp32r
    matmul to emit fp32r). alloc index = flat index + 1.
    """
    nc.scalar.copy(t[:, 0:60], zc[:, 0:60])
    pairs = t[:, 58 : 58 + 57 * 58].rearrange("p (r w) -> p r w", w=58)[:, :, 0:2]
    nc.scalar.copy(pairs, zc[:, 0:114].rearrange("p (r w) -> p r w", w=2))
    nc.scalar.copy(t[:, 3307:ALLOC], zc[:, 0 : ALLOC - 3307])


def build_module(n_img=N_IMG):
    nc = bacc.Bacc()

    x_d = nc.dram_tensor("x", [n_img, C, ALLOC], BF16, kind="ExternalInput")
    w1_d = nc.dram_tensor("w1t", [C, 9 * C], BF16, kind="ExternalInput")
    w2_d = nc.dram_tensor("w2t", [C, 9 * C], BF16, kind="ExternalInput")
    s1_d = nc.dram_tensor("s1", [C, 1], F32, kind="ExternalInput")
    h1_d = nc.dram_tensor("h1", [C, 1], F32, kind="ExternalInput")
    s2_d = nc.dram_tensor("s2", [C, 1], F32, kind="ExternalInput")
    h2_d = nc.dram_tensor("h2", [C, 1], F32, kind="ExternalInput")
    out_d = nc.dram_tensor("out", [n_img, C, HW], F32, kind="ExternalOutput")

    with TileContext(nc) as tc, ExitStack() as ctx:
        wpool = ctx.enter_context(tc.tile_pool(name="wpool", bufs=1))
        xpool = ctx.enter_context(tc.tile_pool(name="xpool", bufs=3))
        o1pool = ctx.enter_context(tc.tile_pool(name="o1pool", bufs=2))
        tmppool = ctx.enter_context(tc.tile_pool(name="tmppool", bufs=4))
        opool = ctx.enter_context(tc.tile_pool(name="opool", bufs=4))
        ps1pool = ctx.enter_context(tc.tile_pool(name="ps1", bufs=4, space="PSUM"))
        ps2pool = ctx.enter_context(tc.tile_pool(name="ps2", bufs=4, space="PSUM"))

        w1_sb = wpool.tile([C, 9 * C], BF16, name="w1_sb")
        w2_sb = wpool.tile([C, 9 * C], BF16, name="w2_sb")
        s1_sb = wpool.tile([C, 1], F32, name="s1_sb")
        h1_sb = wpool.tile([C, 1], F32, name="h1_sb")
        s2_sb = wpool.tile([C, 1], F32, name="s2_sb")
        h2_sb = wpool.tile([C, 1], F32, name="h2_sb")
        zc = wpool.tile([C, 114], F32, name="zc")
        nc.vector.memset(zc[:, :], 0.0)

        def issue_x(img, pieces=2):
            # split the image DMA so the first chunks' matmuls can start
            # before the whole image has landed
            x_pad = xpool.tile([C, ALLOC], BF16, name="x_pad")
            cuts = [ALLOC * k // pieces for k in range(pieces + 1)]
            for a, b in zip(cuts, cuts[1:]):
                nc.sync.dma_start(x_pad[:, a:b], x_d[img, :, a:b])
            return x_pad

        # image 0: first slice (enough for chunk 0's taps) before anything
        # else so the first conv matmul is gated as early as possible
        x_tiles = [None] * n_img
        x0 = xpool.tile([C, ALLOC], BF16, name="x_pad")
        for a, b in ((0, 696), (696, 1586), (1586, 2476), (2476, ALLOC)):
            if a == 696:
                nc.sync.dma_start(w1_sb[:, :], w1_d[:, :])
            nc.sync.dma_start(x0[:, a:b], x_d[0, :, a:b])
        x_tiles[0] = x0
        nc.sync.dma_start(w2_sb[:, :], w2_d[:, :])
        nc.sync.dma_start(s1_sb[:, :], s1_d[:, :])
        nc.sync.dma_start(h1_sb[:, :], h1_d[:, :])
        nc.sync.dma_start(s2_sb[:, :], s2_d[:, :])
        nc.sync.dma_start(h2_sb[:, :], h2_d[:, :])

        # Warm up the PE HAM clock gate during the initial DMA wait: ~3us of
        # throwaway matmuls that depend only on the w1 DMA.
        psw = ps1pool.tile([C, C], F32, name="psw", tag="ps1_t")
        for i in range(14):
            nc.tensor.matmul(
                psw[:, :], w1_sb[:, 0:C], w1_sb[:, 0:C],
                start=(i == 0), stop=(i == 13),
            )

        for img in range(n_img):
            # prefetch next image's input one iteration ahead so it is never
            # queued behind this image's output DMAs
            if img + 1 < n_img:
                x_tiles[img + 1] = issue_x(img + 1)
            x_pad = x_tiles[img]

            # o1_pad is written only by ScalarE: pad zeroing first, then the
            # per-chunk bn+relu writes of the valid columns.
            o1_pad = o1pool.tile([C, ALLOC], BF16, name="o1_pad")
            _zero_pads_scalar(nc, o1_pad, zc)

            # conv1 + bn1 + relu -> o1_pad
            for r0, rows in ROW_CHUNKS:
                nmm = rows * W
                ps = ps1pool.tile([C, nmm], F32, name="ps1_t")
                vbase = (1 + r0) * WP + 2  # alloc index of output (hp=1+r0, wp=1)
                for t in range(9):
                    dh, dw = t // 3 - 1, t % 3 - 1
                    rhs = _valid3(x_pad, vbase + dh * WP + dw, rows)
                    nc.tensor.matmul(
                        ps[:, :].rearrange("p (r w) -> p r w", w=W),
                        w1_sb[:, t * C : (t + 1) * C],
                        rhs,
                        start=(t == 0),
                        stop=(t == 8),
                    )
                nc.scalar.activation(
                    _valid3(o1_pad, vbase, rows),
                    ps[:, :].rearrange("p (r w) -> p r w", w=W),
                    RELU, bias=h1_sb[:, :], scale=s1_sb[:, :],
                )

            # conv2 + bn2 + residual + relu -> out; the final image ends
            # with two half-size chunks so the closing epilogue chain
            # (stt -> act -> dma) after the last matmul is shorter
            chunks2 = ROW_CHUNKS
            if img == n_img - 1:
                chunks2 = ROW_CHUNKS[:-1] + [(48, 4), (52, 4)]
            for r0, rows in chunks2:
                nmm = rows * W
                ps = ps2pool.tile([C, nmm], F32, name="ps2_t")
                vbase = (1 + r0) * WP + 2
                for t in range(9):
                    dh, dw = t // 3 - 1, t % 3 - 1
                    rhs = _valid3(o1_pad, vbase + dh * WP + dw, rows)
                    nc.tensor.matmul(
                        ps[:, :].rearrange("p (r w) -> p r w", w=W),
                        w2_sb[:, t * C : (t + 1) * C],
                        rhs,
                        start=(t == 0),
                        stop=(t == 8),
                    )
                # VectorE: t1 = conv2*scale2 + x (residual), straight from PSUM
                t1 = tmppool.tile([C, nmm], F32, name="t1")
                nc.vector.scalar_tensor_tensor(
                    t1[:, :].rearrange("p (r w) -> p r w", w=W),
                    ps[:, :].rearrange("p (r w) -> p r w", w=W),
                    s2_sb[:, :],
                    _valid3(x_pad, vbase, rows),
                    op0=mybir.AluOpType.mult, op1=mybir.AluOpType.add,
                )
                # ScalarE: out = relu(t1 + shift2)
                outc = opool.tile([C, nmm], F32, name="outc")
                nc.scalar.activation(
                    outc[:, :], t1[:, :], RELU, bias=h2_sb[:, :], scale=1.0
                )
                out_eng = nc.sync if img == n_img - 1 else nc.gpsimd
                out_eng.dma_start(
                    out_d[img, :, r0 * W : r0 * W + nmm], outc[:, :]
                )

    nc.compile()
    return nc


EPS = 1e-5


def _prep_params(w1, g1, b1, m1, v1, w2, g2, b2, m2, v2):
    s1 = (g1 / np.sqrt(v1 + EPS)).astype(np.float32)
    h1 = (b1 - m1 * s1).astype(np.float32)
    s2 = (g2 / np.sqrt(v2 + EPS)).astype(np.float32)
    h2 = (b2 - m2 * s2).astype(np.float32)
    # w[o, i, kh, kw] -> [i, (kh*3+kw)*128 + o]
    import ml_dtypes

    w1t = np.ascontiguousarray(w1.transpose(1, 2, 3, 0).reshape(C, 9 * C)).astype(
        ml_dtypes.bfloat16
    )
    w2t = np.ascontiguousarray(w2.transpose(1, 2, 3, 0).reshape(C, 9 * C)).astype(
        ml_dtypes.bfloat16
    )
    return w1t, w2t, s1.reshape(C, 1), h1.reshape(C, 1), s2.reshape(C, 1), h2.reshape(C, 1)


def pad_images(x):
    """[n, C, 56, 56] -> bf16 [n, C, ALLOC] zero-padded 58x58 + margins."""
    import ml_dtypes

    n = x.shape[0]
    buf = np.zeros((n, C, ALLOC), dtype=ml_dtypes.bfloat16)
    v = buf[:, :, 60 : 60 + 58 * 56].reshape(n, C, 56, 58)
    v[:, :, :, :56] = x.astype(ml_dtypes.bfloat16)
    return buf


def kernel(x, w1, g1, b1, m1, v1, w2, g2, b2, m2, v2):
    x = np.asarray(x, dtype=np.float32)
    n = x.shape[0]
    assert n == N_CORES * N_IMG, x.shape
    w1t, w2t, s1, h1, s2, h2 = _prep_params(
        np.asarray(w1), np.asarray(g1), np.asarray(b1), np.asarray(m1), np.asarray(v1),
        np.asarray(w2), np.asarray(g2), np.asarray(b2), np.asarray(m2), np.asarray(v2),
    )
    xp = pad_images(x.reshape(n, C, H, W))
    nc = build_module()
    in_maps = []
    for cid in range(N_CORES):
        xs = np.ascontiguousarray(xp[cid * N_IMG : (cid + 1) * N_IMG])
        in_maps.append(
            {"x": xs, "w1t": w1t, "w2t": w2t, "s1": s1, "h1": h1, "s2": s2, "h2": h2}
        )
    res = run_bass_kernel_spmd(nc, in_maps, core_ids=list(range(N_CORES)))
    out = np.concatenate([r["out"] for r in res.results], axis=0)
    return out.reshape(n, C, H, W)


# revision 16
# speedup vs baseline: 1.0043x; 1.0043x over previous
"""ResNet BasicBlock (conv3x3-bn-relu-conv3x3-bn-add-relu) on 8 TRN2 cores.

Data-parallel: batch N=64 split into 8 images per core; conv/BN params
replicated. Each 3x3 conv is computed as 9 shifted [128ci x 128co] fp32r
matmuls accumulated in PSUM over a zero-padded [C, 58*58] SBUF image layout
(channels on partitions, padded spatial flattened on the free dim). The host
pre-pads x so each image is one contiguous DMA. Matmul rhs uses strided
valid-column access patterns so no PE cycles are spent on pad columns.
"""

import numpy as np
from contextlib import ExitStack

import concourse.bass as bass
import concourse.bacc as bacc
import concourse.mybir as mybir
from concourse.tile import TileContext
from concourse.bass_utils import run_bass_kernel_spmd

F32 = mybir.dt.float32
F32R = mybir.dt.float32r
BF16 = mybir.dt.bfloat16
RELU = mybir.ActivationFunctionType.Relu
IDENT = mybir.ActivationFunctionType.Identity

N_CORES = 8
N_IMG = 8          # images per core
C = 128            # channels (== partitions)
H = W = 56
HP = WP = 58       # padded spatial
S = HP * WP        # 3364 padded flat size
ALLOC = S + 4      # margins so strided valid-col views stay in bounds
HW = H * W         # 3136
ROW_CHUNKS = [(8 * k, 8) for k in range(7)]  # (start row, rows); 448 <= 512
NMAX = 8 * W


def _valid3(t, start, rows):
    """3D [C, rows, 56] valid-column view of padded tile t at alloc offset
    `start` (the alloc index of the first element of the window)."""
    return t[:, start : start + 58 * rows].rearrange("p (r w) -> p r w", w=58)[
        :, :, 0:56
    ]


def _zero_pads_scalar(nc, t, zc):
    """Zero every padded position of a [128, ALLOC] image tile on ScalarE.

    Implemented as ACT copies from a zero constant tile so the output dtype
    stays float32r (the BIR verifier requires every producer feeding an fp32r
    matmul to emit fp32r). alloc index = flat index + 1.
    """
    nc.scalar.copy(t[:, 0:60], zc[:, 0:60])
    pairs = t[:, 58 : 58 + 57 * 58].rearrange("p (r w) -> p r w", w=58)[:, :, 0:2]
    nc.scalar.copy(pairs, zc[:, 0:114].rearrange("p (r w) -> p r w", w=2))
    nc.scalar.copy(t[:, 3307:ALLOC], zc[:, 0 : ALLOC - 3307])


def build_module(n_img=N_IMG):
    nc = bacc.Bacc()

    x_d = nc.dram_tensor("x", [n_img, C, ALLOC], BF16, kind="ExternalInput")
    w1_d = nc.dram_tensor("w1t", [C, 9 * C], BF16, kind="ExternalInput")
    w2_d = nc.dram_tensor("w2t", [C, 9 * C], BF16, kind="ExternalInput")
    s1_d = nc.dram_tensor("s1", [C, 1], F32, kind="ExternalInput")
    h1_d = nc.dram_tensor("h1", [C, 1], F32, kind="ExternalInput")
    s2_d = nc.dram_tensor("s2", [C, 1], F32, kind="ExternalInput")
    h2_d = nc.dram_tensor("h2", [C, 1], F32, kind="ExternalInput")
    out_d = nc.dram_tensor("out", [n_img, C, HW], F32, kind="ExternalOutput")

    with TileContext(nc) as tc, ExitStack() as ctx:
        wpool = ctx.enter_context(tc.tile_pool(name="wpool", bufs=1))
        xpool = ctx.enter_context(tc.tile_pool(name="xpool", bufs=3))
        o1pool = ctx.enter_context(tc.tile_pool(name="o1pool", bufs=2))
        tmppool = ctx.enter_context(tc.tile_pool(name="tmppool", bufs=4))
        opool = ctx.enter_context(tc.tile_pool(name="opool", bufs=4))
        ps1pool = ctx.enter_context(tc.tile_pool(name="ps1", bufs=4, space="PSUM"))
        ps2pool = ctx.enter_context(tc.tile_pool(name="ps2", bufs=4, space="PSUM"))

        w1_sb = wpool.tile([C, 9 * C], BF16, name="w1_sb")
        w2_sb = wpool.tile([C, 9 * C], BF16, name="w2_sb")
        s1_sb = wpool.tile([C, 1], F32, name="s1_sb")
        h1_sb = wpool.tile([C, 1], F32, name="h1_sb")
        s2_sb = wpool.tile([C, 1], F32, name="s2_sb")
        h2_sb = wpool.tile([C, 1], F32, name="h2_sb")
        zc = wpool.tile([C, 114], F32, name="zc")
        nc.vector.memset(zc[:, :], 0.0)
        nc.sync.dma_start(w1_sb[:, :], w1_d[:, :])

        def issue_x(img, pieces=2):
            # split the image DMA so the first chunks' matmuls can start
            # before the whole image has landed
            x_pad = xpool.tile([C, ALLOC], BF16, name="x_pad")
            cuts = [ALLOC * k // pieces for k in range(pieces + 1)]
            for a, b in zip(cuts, cuts[1:]):
                nc.sync.dma_start(x_pad[:, a:b], x_d[img, :, a:b])
            return x_pad

        x_tiles = [None] * n_img
        x_tiles[0] = issue_x(0, pieces=4)
        nc.sync.dma_start(w2_sb[:, :], w2_d[:, :])
        nc.sync.dma_start(s1_sb[:, :], s1_d[:, :])
        nc.sync.dma_start(h1_sb[:, :], h1_d[:, :])
        nc.sync.dma_start(s2_sb[:, :], s2_d[:, :])
        nc.sync.dma_start(h2_sb[:, :], h2_d[:, :])

        # Warm up the PE HAM clock gate during the initial DMA wait: ~3us of
        # throwaway matmuls that depend only on the w1 DMA.
        psw = ps1pool.tile([C, C], F32, name="psw", tag="ps1_t")
        for i in range(14):
            nc.tensor.matmul(
                psw[:, :], w1_sb[:, 0:C], w1_sb[:, 0:C],
                start=(i == 0), stop=(i == 13),
            )

        for img in range(n_img):
            # prefetch next image's input one iteration ahead so it is never
            # queued behind this image's output DMAs
            if img + 1 < n_img:
                x_tiles[img + 1] = issue_x(img + 1)
            x_pad = x_tiles[img]

            # o1_pad is written only by ScalarE: pad zeroing first, then the
            # per-chunk bn+relu writes of the valid columns.
            o1_pad = o1pool.tile([C, ALLOC], BF16, name="o1_pad")
            _zero_pads_scalar(nc, o1_pad, zc)

            # conv1 + bn1 + relu -> o1_pad
            for r0, rows in ROW_CHUNKS:
                nmm = rows * W
                ps = ps1pool.tile([C, nmm], F32, name="ps1_t")
                vbase = (1 + r0) * WP + 2  # alloc index of output (hp=1+r0, wp=1)
                for t in range(9):
                    dh, dw = t // 3 - 1, t % 3 - 1
                    rhs = _valid3(x_pad, vbase + dh * WP + dw, rows)
                    nc.tensor.matmul(
                        ps[:, :].rearrange("p (r w) -> p r w", w=W),
                        w1_sb[:, t * C : (t + 1) * C],
                        rhs,
                        start=(t == 0),
                        stop=(t == 8),
                    )
                nc.scalar.activation(
                    _valid3(o1_pad, vbase, rows),
                    ps[:, :].rearrange("p (r w) -> p r w", w=W),
                    RELU, bias=h1_sb[:, :], scale=s1_sb[:, :],
                )

            # conv2 + bn2 + residual + relu -> out
            for r0, rows in ROW_CHUNKS:
                nmm = rows * W
                ps = ps2pool.tile([C, nmm], F32, name="ps2_t")
                vbase = (1 + r0) * WP + 2
                for t in range(9):
                    dh, dw = t // 3 - 1, t % 3 - 1
                    rhs = _valid3(o1_pad, vbase + dh * WP + dw, rows)
                    nc.tensor.matmul(
                        ps[:, :].rearrange("p (r w) -> p r w", w=W),
                        w2_sb[:, t * C : (t + 1) * C],
                        rhs,
                        start=(t == 0),
                        stop=(t == 8),
                    )
                # VectorE: t1 = conv2*scale2 + x (residual), straight from PSUM
                t1 = tmppool.tile([C, nmm], F32, name="t1")
                nc.vector.scalar_tensor_tensor(
                    t1[:, :].rearrange("p (r w) -> p r w", w=W),
                    ps[:, :].rearrange("p (r w) -> p r w", w=W),
                    s2_sb[:, :],
                    _valid3(x_pad, vbase, rows),
                    op0=mybir.AluOpType.mult, op1=mybir.AluOpType.add,
                )
                # ScalarE: out = relu(t1 + shift2)
                outc = opool.tile([C, nmm], F32, name="outc")
                nc.scalar.activation(
                    outc[:, :], t1[:, :], RELU, bias=h2_sb[:, :], scale=1.0
                )
                out_eng = nc.sync if img == n_img - 1 else nc.gpsimd
                out_eng.dma_start(
                    out_d[img, :, r0 * W : r0 * W + nmm], outc[:, :]
                )

    nc.compile()
    return nc


EPS = 1e-5


def _prep_params(w1, g1, b1, m1, v1, w2, g2, b2, m2, v2):
    s1 = (g1 / np.sqrt(v1 + EPS)).astype(np.float32)
    h1 = (b1 - m1 * s1).astype(np.float32)
    s2 = (g2 / np.sqrt(v2 + EPS)).astype(np.float32)
    h2 = (b2 - m2 * s2).astype(np.float32)
    # w[o, i, kh, kw] -> [i, (kh*3+kw)*128 + o]
    import ml_dtypes

    w1t = np.ascontiguousarray(w1.transpose(1, 2, 3, 0).reshape(C, 9 * C)).astype(
        ml_dtypes.bfloat16
    )
    w2t = np.ascontiguousarray(w2.transpose(1, 2, 3, 0).reshape(C, 9 * C)).astype(
        ml_dtypes.bfloat16
    )
    return w1t, w2t, s1.reshape(C, 1), h1.reshape(C, 1), s2.reshape(C, 1), h2.reshape(C, 1)


def pad_images(x):
    """[n, C, 56, 56] -> bf16 [n, C, ALLOC] zero-padded 58x58 + margins."""
    import ml_dtypes

    n = x.shape[0]
    buf = np.zeros((n, C, ALLOC), dtype=ml_dtypes.bfloat16)
    v = buf[:, :, 60 : 60 + 58 * 56].reshape(n, C, 56, 58)
    v[:, :, :, :56] = x.astype(ml_dtypes.bfloat16)
    return buf


def kernel(x, w1, g1, b1, m1, v1, w2, g2, b2, m2, v2):
    x = np.asarray(x, dtype=np.float32)
    n = x.shape[0]
    assert n == N_CORES * N_IMG, x.shape
    w1t, w2t, s1, h1, s2, h2 = _prep_params(
        np.asarray(w1), np.asarray(g1), np.asarray(b1), np.asarray(m1), np.asarray(v1),
        np.asarray(w2), np.asarray(g2), np.asarray(b2), np.asarray(m2), np.asarray(v2),
    )
    xp = pad_images(x.reshape(n, C, H, W))
    nc = build_module()
    in_maps = []
    for cid in range(N_CORES):
        xs = np.ascontiguousarray(xp[cid * N_IMG : (cid + 1) * N_IMG])
        in_maps.append(
            {"x": xs, "w1t": w1t, "w2t": w2t, "s1": s1, "h1": h1, "s2": s2, "h2": h2}
        )
    res = run_bass_kernel_spmd(nc, in_maps, core_ids=list(range(N_CORES)))
    out = np.concatenate([r["out"] for r in res.results], axis=0)
    return out.reshape(n, C, H, W)


# revision 17
# speedup vs baseline: 1.0056x; 1.0013x over previous
"""ResNet BasicBlock (conv3x3-bn-relu-conv3x3-bn-add-relu) on 8 TRN2 cores.

Data-parallel: batch N=64 split into 8 images per core; conv/BN params
replicated. Each 3x3 conv is computed as 9 shifted [128ci x 128co] fp32r
matmuls accumulated in PSUM over a zero-padded [C, 58*58] SBUF image layout
(channels on partitions, padded spatial flattened on the free dim). The host
pre-pads x so each image is one contiguous DMA. Matmul rhs uses strided
valid-column access patterns so no PE cycles are spent on pad columns.
"""

import numpy as np
from contextlib import ExitStack

import concourse.bass as bass
import concourse.bacc as bacc
import concourse.mybir as mybir
from concourse.tile import TileContext
from concourse.bass_utils import run_bass_kernel_spmd

F32 = mybir.dt.float32
F32R = mybir.dt.float32r
BF16 = mybir.dt.bfloat16
RELU = mybir.ActivationFunctionType.Relu
IDENT = mybir.ActivationFunctionType.Identity

N_CORES = 8
N_IMG = 8          # images per core
C = 128            # channels (== partitions)
H = W = 56
HP = WP = 58       # padded spatial
S = HP * WP        # 3364 padded flat size
ALLOC = S + 4      # margins so strided valid-col views stay in bounds
HW = H * W         # 3136
ROW_CHUNKS = [(8 * k, 8) for k in range(7)]  # (start row, rows); 448 <= 512
NMAX = 8 * W


def _valid3(t, start, rows):
    """3D [C, rows, 56] valid-column view of padded tile t at alloc offset
    `start` (the alloc index of the first element of the window)."""
    return t[:, start : start + 58 * rows].rearrange("p (r w) -> p r w", w=58)[
        :, :, 0:56
    ]


def _zero_pads_scalar(nc, t, zc):
    """Zero every padded position of a [128, ALLOC] image tile on ScalarE.

    Implemented as ACT copies from a zero constant tile so the output dtype
    stays float32r (the BIR verifier requires every producer feeding an fp32r
    matmul to emit fp32r). alloc index = flat index + 1.
    """
    nc.scalar.copy(t[:, 0:60], zc[:, 0:60])
    pairs = t[:, 58 : 58 + 57 * 58].rearrange("p (r w) -> p r w", w=58)[:, :, 0:2]
    nc.scalar.copy(pairs, zc[:, 0:114].rearrange("p (r w) -> p r w", w=2))
    nc.scalar.copy(t[:, 3307:ALLOC], zc[:, 0 : ALLOC - 3307])


def build_module(n_img=N_IMG):
    nc = bacc.Bacc()

    x_d = nc.dram_tensor("x", [n_img, C, ALLOC], BF16, kind="ExternalInput")
    w1_d = nc.dram_tensor("w1t", [C, 9 * C], BF16, kind="ExternalInput")
    w2_d = nc.dram_tensor("w2t", [C, 9 * C], BF16, kind="ExternalInput")
    s1_d = nc.dram_tensor("s1", [C, 1], F32, kind="ExternalInput")
    h1_d = nc.dram_tensor("h1", [C, 1], F32, kind="ExternalInput")
    s2_d = nc.dram_tensor("s2", [C, 1], F32, kind="ExternalInput")
    h2_d = nc.dram_tensor("h2", [C, 1], F32, kind="ExternalInput")
    out_d = nc.dram_tensor("out", [n_img, C, HW], F32, kind="ExternalOutput")

    with TileContext(nc) as tc, ExitStack() as ctx:
        wpool = ctx.enter_context(tc.tile_pool(name="wpool", bufs=1))
        xpool = ctx.enter_context(tc.tile_pool(name="xpool", bufs=4))
        o1pool = ctx.enter_context(tc.tile_pool(name="o1pool", bufs=3))
        tmppool = ctx.enter_context(tc.tile_pool(name="tmppool", bufs=6))
        opool = ctx.enter_context(tc.tile_pool(name="opool", bufs=6))
        ps1pool = ctx.enter_context(tc.tile_pool(name="ps1", bufs=4, space="PSUM"))
        ps2pool = ctx.enter_context(tc.tile_pool(name="ps2", bufs=4, space="PSUM"))

        w1_sb = wpool.tile([C, 9 * C], BF16, name="w1_sb")
        w2_sb = wpool.tile([C, 9 * C], BF16, name="w2_sb")
        s1_sb = wpool.tile([C, 1], F32, name="s1_sb")
        h1_sb = wpool.tile([C, 1], F32, name="h1_sb")
        s2_sb = wpool.tile([C, 1], F32, name="s2_sb")
        h2_sb = wpool.tile([C, 1], F32, name="h2_sb")
        zc = wpool.tile([C, 114], F32, name="zc")
        nc.vector.memset(zc[:, :], 0.0)
        nc.sync.dma_start(w1_sb[:, :], w1_d[:, :])

        def issue_x(img, pieces=2):
            # split the image DMA so the first chunks' matmuls can start
            # before the whole image has landed
            x_pad = xpool.tile([C, ALLOC], BF16, name="x_pad")
            cuts = [ALLOC * k // pieces for k in range(pieces + 1)]
            for a, b in zip(cuts, cuts[1:]):
                nc.sync.dma_start(x_pad[:, a:b], x_d[img, :, a:b])
            return x_pad

        x_tiles = [None] * n_img
        x_tiles[0] = issue_x(0, pieces=4)
        nc.sync.dma_start(w2_sb[:, :], w2_d[:, :])
        nc.sync.dma_start(s1_sb[:, :], s1_d[:, :])
        nc.sync.dma_start(h1_sb[:, :], h1_d[:, :])
        nc.sync.dma_start(s2_sb[:, :], s2_d[:, :])
        nc.sync.dma_start(h2_sb[:, :], h2_d[:, :])

        # Warm up the PE HAM clock gate during the initial DMA wait: ~3us of
        # throwaway matmuls that depend only on the w1 DMA.
        psw = ps1pool.tile([C, C], F32, name="psw", tag="ps1_t")
        for i in range(14):
            nc.tensor.matmul(
                psw[:, :], w1_sb[:, 0:C], w1_sb[:, 0:C],
                start=(i == 0), stop=(i == 13),
            )

        for img in range(n_img):
            # prefetch next image's input one iteration ahead so it is never
            # queued behind this image's output DMAs
            if img + 1 < n_img:
                x_tiles[img + 1] = issue_x(img + 1)
            x_pad = x_tiles[img]

            # o1_pad is written only by ScalarE: pad zeroing first, then the
            # per-chunk bn+relu writes of the valid columns.
            o1_pad = o1pool.tile([C, ALLOC], BF16, name="o1_pad")
            _zero_pads_scalar(nc, o1_pad, zc)

            # conv1 + bn1 + relu -> o1_pad
            for r0, rows in ROW_CHUNKS:
                nmm = rows * W
                ps = ps1pool.tile([C, nmm], F32, name="ps1_t")
                vbase = (1 + r0) * WP + 2  # alloc index of output (hp=1+r0, wp=1)
                for t in range(9):
                    dh, dw = t // 3 - 1, t % 3 - 1
                    rhs = _valid3(x_pad, vbase + dh * WP + dw, rows)
                    nc.tensor.matmul(
                        ps[:, :].rearrange("p (r w) -> p r w", w=W),
                        w1_sb[:, t * C : (t + 1) * C],
                        rhs,
                        start=(t == 0),
                        stop=(t == 8),
                    )
                nc.scalar.activation(
                    _valid3(o1_pad, vbase, rows),
                    ps[:, :].rearrange("p (r w) -> p r w", w=W),
                    RELU, bias=h1_sb[:, :], scale=s1_sb[:, :],
                )

            # conv2 + bn2 + residual + relu -> out
            for r0, rows in ROW_CHUNKS:
                nmm = rows * W
                ps = ps2pool.tile([C, nmm], F32, name="ps2_t")
                vbase = (1 + r0) * WP + 2
                for t in range(9):
                    dh, dw = t // 3 - 1, t % 3 - 1
                    rhs = _valid3(o1_pad, vbase + dh * WP + dw, rows)
                    nc.tensor.matmul(
                        ps[:, :].rearrange("p (r w) -> p r w", w=W),
                        w2_sb[:, t * C : (t + 1) * C],
                        rhs,
                        start=(t == 0),
                        stop=(t == 8),
                    )
                # VectorE: t1 = conv2*scale2 + x (residual), straight from PSUM
                t1 = tmppool.tile([C, nmm], F32, name="t1")
                nc.vector.scalar_tensor_tensor(
                    t1[:, :].rearrange("p (r w) -> p r w", w=W),
                    ps[:, :].rearrange("p (r w) -> p r w", w=W),
                    s2_sb[:, :],
                    _valid3(x_pad, vbase, rows),
                    op0=mybir.AluOpType.mult, op1=mybir.AluOpType.add,
                )
                # ScalarE: out = relu(t1 + shift2)
                outc = opool.tile([C, nmm], F32, name="outc")
                nc.scalar.activation(
                    outc[:, :], t1[:, :], RELU, bias=h2_sb[:, :], scale=1.0
                )
                out_eng = nc.sync if img == n_img - 1 else nc.gpsimd
                out_eng.dma_start(
                    out_d[img, :, r0 * W : r0 * W + nmm], outc[:, :]
                )

    nc.compile()
    return nc


EPS = 1e-5


def _prep_params(w1, g1, b1, m1, v1, w2, g2, b2, m2, v2):
    s1 = (g1 / np.sqrt(v1 + EPS)).astype(np.float32)
    h1 = (b1 - m1 * s1).astype(np.float32)
    s2 = (g2 / np.sqrt(v2 + EPS)).astype(np.float32)
    h2 = (b2 - m2 * s2).astype(np.float32)
    # w[o, i, kh, kw] -> [i, (kh*3+kw)*128 + o]
    import ml_dtypes

    w1t = np.ascontiguousarray(w1.transpose(1, 2, 3, 0).reshape(C, 9 * C)).astype(
        ml_dtypes.bfloat16
    )
    w2t = np.ascontiguousarray(w2.transpose(1, 2, 3, 0).reshape(C, 9 * C)).astype(
        ml_dtypes.bfloat16
    )
    return w1t, w2t, s1.reshape(C, 1), h1.reshape(C, 1), s2.reshape(C, 1), h2.reshape(C, 1)


def pad_images(x):
    """[n, C, 56, 56] -> bf16 [n, C, ALLOC] zero-padded 58x58 + margins."""
    import ml_dtypes

    n = x.shape[0]
    buf = np.zeros((n, C, ALLOC), dtype=ml_dtypes.bfloat16)
    v = buf[:, :, 60 : 60 + 58 * 56].reshape(n, C, 56, 58)
    v[:, :, :, :56] = x.astype(ml_dtypes.bfloat16)
    return buf


def kernel(x, w1, g1, b1, m1, v1, w2, g2, b2, m2, v2):
    x = np.asarray(x, dtype=np.float32)
    n = x.shape[0]
    assert n == N_CORES * N_IMG, x.shape
    w1t, w2t, s1, h1, s2, h2 = _prep_params(
        np.asarray(w1), np.asarray(g1), np.asarray(b1), np.asarray(m1), np.asarray(v1),
        np.asarray(w2), np.asarray(g2), np.asarray(b2), np.asarray(m2), np.asarray(v2),
    )
    xp = pad_images(x.reshape(n, C, H, W))
    nc = build_module()
    in_maps = []
    for cid in range(N_CORES):
        xs = np.ascontiguousarray(xp[cid * N_IMG : (cid + 1) * N_IMG])
        in_maps.append(
            {"x": xs, "w1t": w1t, "w2t": w2t, "s1": s1, "h1": h1, "s2": s2, "h2": h2}
        )
    res = run_bass_kernel_spmd(nc, in_maps, core_ids=list(range(N_CORES)))
    out = np.concatenate([r["out"] for r in res.results], axis=0)
    return out.reshape(n, C, H, W)
